# revision 1
# baseline (speedup 1.0000x reference)
"""Trainium2 Bass kernel for nn_EndtoEndIntervetionMap.

Computes, for B=4,194,304 rows split evenly over 8 NeuronCores:
    beta = sigmoid(relu(x @ W1 + b1) @ W2 + b2)          (tiny MLP, per row)
    14 explicit-Euler SIR steps on y=(S,I,R) with that beta.

Design (per core, R rows):
  MLP runs in a "transposed" layout (rows along the free dim):
    - x loaded as x_nat[128, 8*A] with partition p holding rows {128a+p}.
    - PE transpose turns [128,128] x-blocks into xT with the 8 features on
      partitions; chunklet-pairs land at 32-aligned partition bases so a
      K=16 block-diagonal [W1;W1] matmul (tile_position rows 0/32/64/96)
      computes h^T for 2 chunklets (256 rows) per 512-col stream.
    - ScalarE applies bias+relu evacuating PSUM->SBUF.
    - mm2 packs [W2;W2] as [128,2] and col-tiles 4 matmuls into ONE PSUM
      bank (out partitions {32q,32q+1}), so one full-bank copy evacuates
      beta_pre for 4096 rows.
    - A strided SBUF->SBUF DMA reshapes beta_pre strips to a partition-major
      [128, *] layout (512B contiguous runs on both sides).
  SIR runs partition-major on VectorE in scaled coordinates u=t*S, v=t*I
  (t = beta/2): p=u*v; u-=p; v=0.95*v+p  (3 fused DVE ops per step).
  Final S=u/t, I=v/t, R=1-S-I written back strided into the interleaved
  [*,3] layout and stored with one contiguous DMA.
"""

import os
import sys

import numpy as np

for _p in ("/opt/trn_rl_repo",):
    if _p not in sys.path:
        sys.path.insert(0, _p)

import concourse.bass as bass
import concourse.mybir as mybir
from concourse import bacc
from concourse.bass_utils import run_bass_kernel_spmd
from concourse.tile import TileContext

F32 = mybir.dt.float32
AF = mybir.ActivationFunctionType
OP = mybir.AluOpType

N_CORES = 8
GAMMA = 0.1
STEPS = 2
WINDOW = 7
N_ITER = WINDOW * STEPS  # 14
DT = 1.0 / STEPS  # 0.5
CDEC = 1.0 - DT * GAMMA  # 0.95

_NC_CACHE = {}


def build_nc(RC: int, ST: int = 16384, GRP: int = 8, repeat: int = 1, sim_safe: bool = False, trace_sim: bool = False, ablate: str = ''):
    """Build the single-core Bass program for RC rows.

    RC must be divisible by ST*GRP; ST must be divisible by 16384's
    structure: ST = 128 partitions * A rows with A divisible by 16.
    repeat re-emits the whole pipeline N times (for timing: the dispatch
    overhead through axon dwarfs one iteration, so per-iteration time is
    measured as (T(repeat=R) - T(repeat=1)) / (R - 1)).
    """
    A = ST // 128  # rows per partition in x_nat / chunklets per ST
    NB = A // 16  # x-blocks (of 16 chunklets) per ST
    NST = RC // ST
    NGRP = NST // GRP
    assert RC == NST * ST and NST == NGRP * GRP and A % 16 == 0
    assert GRP % 4 == 0, "quad-batched beta staging needs GRP % 4 == 0"

    nc = bacc.Bacc(None, target_bir_lowering=False)

    x_d = nc.declare_dram_parameter("x", [RC, 8], F32, isOutput=False)
    y_d = nc.declare_dram_parameter("y", [RC, 3], F32, isOutput=False)
    w1s_d = nc.declare_dram_parameter("w1s", [128, 128], F32, isOutput=False)
    w2p_d = nc.declare_dram_parameter("w2p", [128, 32], F32, isOutput=False)
    b1p_d = nc.declare_dram_parameter("b1p", [128, 1], F32, isOutput=False)
    b2b_d = nc.declare_dram_parameter("b2b", [128, 1], F32, isOutput=False)
    id_d = nc.declare_dram_parameter("ident", [128, 128], F32, isOutput=False)
    yo_d = nc.declare_dram_parameter("yout", [RC, 3], F32, isOutput=True)

    XCOLS = 8 * A  # real x columns per partition
    XPAD = 16  # padding columns read by the last shifted transpose

    with TileContext(nc, trace_sim=trace_sim) as tc:
        with (
            tc.tile_pool(name="consts", bufs=1) as cpool,
            tc.tile_pool(name="xnat", bufs=3) as xpool,
            tc.tile_pool(name="xtsb", bufs=2) as xtpool,
            tc.tile_pool(name="htsb", bufs=4) as hspool,
            tc.tile_pool(name="sgb", bufs=1) as sbpool,
            tc.tile_pool(name="bpm", bufs=2) as bpool,
            tc.tile_pool(name="bpm4p", bufs=1) as b4pool,
            tc.tile_pool(name="sir", bufs=1) as spool,
            tc.tile_pool(name="ynat", bufs=2) as ypool,
            tc.tile_pool(name="ps_xtt", bufs=2, space="PSUM") as ptpool,
            tc.tile_pool(name="ps_ht", bufs=2, space="PSUM") as phpool,
            tc.tile_pool(name="ps_bb", bufs=2, space="PSUM") as pbpool,
        ):
            w1s = cpool.tile([128, 128], F32)
            w2p = cpool.tile([128, 32], F32)
            b1p = cpool.tile([128, 1], F32)
            b2b = cpool.tile([128, 1], F32)
            ident = cpool.tile([128, 128], F32)
            nc.sync.dma_start(w1s[:], w1s_d[:])
            nc.sync.dma_start(w2p[:], w2p_d[:])
            nc.sync.dma_start(b1p[:], b1p_d[:])
            nc.sync.dma_start(b2b[:], b2b_d[:])
            nc.sync.dma_start(ident[:], id_d[:])

            for g in range(NGRP * repeat):
                g = g % NGRP
                gbase = g * GRP * ST
                W = GRP * A  # free width of partition-major group tiles
                b_pm = bpool.tile([128, W], F32)
                # Staging tile for the beta reshape. DMA descriptors only
                # honor a single (leading) partition dim per side, so the
                # strips first land here with the cbj axis along the free
                # dim, then four [32, W] partition-range copies spread them
                # to b_pm. The strided writes cover every cell; the memset
                # is only for the simulator's init tracker.
                bpm4 = b4pool.tile([32, 4 * W], F32, tag="bpm4")
                if sim_safe:
                    nc.vector.memset(bpm4[:], 0.0)

                for stl in range(GRP):
                    stbase = gbase + stl * ST

                    x_nat = xpool.tile([128, XCOLS + XPAD], F32)
                    nc.vector.memset(x_nat[:, XCOLS : XCOLS + XPAD], 0.0)
                    nc.scalar.dma_start(
                        x_nat[:, 0:XCOLS],
                        x_d[stbase : stbase + ST, :].rearrange(
                            "(a p) k -> p a k", p=128
                        ),
                    )

                    xt_sb = xtpool.tile([128, 256 * NB], F32)
                    for b in range(NB):
                        xtt = ptpool.tile([128, 256], F32)
                        if "pe" not in ablate:
                            nc.tensor.transpose(
                                xtt[:, 0:128],
                                x_nat[:, 128 * b : 128 * b + 128],
                                ident[:],
                            )
                            nc.tensor.transpose(
                                xtt[:, 128:256],
                                x_nat[:, 128 * b + 16 : 128 * b + 144],
                                ident[:],
                            )
                        nc.scalar.copy(xt_sb[:, 256 * b : 256 * b + 256], xtt[:])

                    if stl % 4 == 0:
                        sgb2 = []
                        for B2 in range(NB // 2):
                            sg_t = sbpool.tile(
                                [128, 2048], F32, tag=f"sg{B2}", name=f"sg{B2}"
                            )
                            sgb2.append(sg_t)
                    stq = stl % 4

                    xt_cb = xt_sb[:].rearrange("r (cb p) -> r cb p", p=128)
                    for B2 in range(NB // 2):
                        bbank = pbpool.tile([128, 512], F32)
                        for qp in range(2):
                            # two matmuls fill a 2-bank PSUM tile; one
                            # activation drains both (fewer, larger ACT ops)
                            hT = phpool.tile([128, 1024], F32)
                            hT_sb = hspool.tile([128, 1024], F32)
                            for qh in range(2):
                                q = 2 * qp + qh
                                if "pe" not in ablate:
                                    nc.tensor.matmul(
                                        hT[:, 512 * qh : 512 * qh + 512],
                                        w1s[32 * q : 32 * q + 16, :],
                                        xt_cb[32 * q : 32 * q + 16, B2 :: 4, :],
                                        tile_position=(32 * q, 0),
                                    )
                            if "relu" not in ablate:
                                nc.scalar.activation(
                                    hT_sb[:],
                                    hT[:],
                                    AF.Relu,
                                    bias=b1p[:, 0:1],
                                    scale=1.0,
                                )
                            for qh in range(2):
                                q = 2 * qp + qh
                                if "pe" not in ablate:
                                    nc.tensor.matmul(
                                        bbank[32 * q : 32 * q + 32, :],
                                        w2p[:],
                                        hT_sb[:, 512 * qh : 512 * qh + 512],
                                        tile_position=(0, 32 * q),
                                    )
                        # stage this ST's bank into the quad tile (cb-strided)
                        nc.vector.tensor_copy(
                            sgb2[B2][:, :].rearrange(
                                "r (cb s p) -> r cb s p", cb=4, s=4
                            )[:, :, stq, :],
                            bbank[:].rearrange("r (cb p) -> r cb p", p=128),
                        )

                    if stl % 4 == 3:
                        h4 = stl // 4
                        for B2 in range(NB // 2):
                            coff = 16 * (B2 // 2) + 2 * (B2 % 2)
                            for q in range(4):
                                src = sgb2[B2][
                                    32 * q : 32 * q + 2, :
                                ].rearrange("c (cb sp) -> c cb sp", cb=4)
                                dst = bpm4[
                                    coff + 4 * q : coff + 4 * q + 2, :
                                ].rearrange("c (cb w) -> c cb w", cb=4)[
                                    :, :, 512 * h4 : 512 * h4 + 512
                                ]
                                nc.sync.dma_start(dst, src)

                # spread the staged beta to the partition-major layout
                for cbj in range(4):
                    nc.sync.dma_start(
                        b_pm[32 * cbj : 32 * cbj + 32, :],
                        bpm4[:, cbj * W : (cbj + 1) * W],
                    )

                # ---- SIR phase for this group ----
                GR = GRP * ST  # rows in group
                t_pm = spool.tile([128, W], F32, tag="t_pm")
                nc.scalar.activation(t_pm[:], b_pm[:], AF.Sigmoid, bias=b2b[:, 0:1])
                th = spool.tile([128, W], F32, tag="th")
                nc.vector.tensor_scalar_mul(th[:], t_pm[:], DT)
                r2 = spool.tile([128, W], F32, tag="r2")
                nc.vector.reciprocal(r2[:], th[:])

                y_nat = ypool.tile([128, 3 * W], F32)
                nc.scalar.dma_start(
                    y_nat[:],
                    y_d[gbase : gbase + GR, :].rearrange(
                        "(s p t) c -> p s t c", s=GRP, p=128
                    ),
                )
                y3 = y_nat[:].rearrange("p (t c) -> p t c", c=3)
                Sap = y3[:, :, 0]
                Iap = y3[:, :, 1]
                Rap = y3[:, :, 2]

                # SIR steps in sign-tracked coordinates: after the two
                # transitional steps, (m, n) = (u, -v) is a fixed point of
                #   m' = (n + 1) * m ;  n' = (m + c) * n
                # so each Euler step is TWO fused scalar_tensor_tensor ops
                # (ping-pong buffers; both read the old pair).
                mA = spool.tile([128, W], F32, tag="u")
                nA = spool.tile([128, W], F32, tag="v")
                mB = spool.tile([128, W], F32, tag="u2")
                nB = spool.tile([128, W], F32, tag="v2")
                pte = spool.tile([128, W], F32, tag="pte")
                nc.vector.tensor_mul(mA[:], th[:], Sap)
                nc.vector.tensor_mul(nA[:], th[:], Iap)
                n_it = 2 if "sir" in ablate else N_ITER
                cur_m, cur_n, alt_m, alt_n = mA, nA, mB, nB
                for k in range(n_it):
                    op_m = OP.subtract if k < 2 else OP.add
                    op_n = OP.subtract if k == 1 else OP.add
                    nc.vector.scalar_tensor_tensor(
                        alt_m[:], cur_n[:], 1.0, cur_m[:], op_m, OP.mult
                    )
                    nc.vector.scalar_tensor_tensor(
                        alt_n[:], cur_m[:], CDEC, cur_n[:], op_n, OP.mult
                    )
                    cur_m, cur_n, alt_m, alt_n = alt_m, alt_n, cur_m, cur_n
                nc.vector.tensor_mul(Sap, cur_m[:], r2[:])
                nc.vector.scalar_tensor_tensor(
                    Iap, cur_n[:], -1.0, r2[:], OP.mult, OP.mult
                )
                nc.vector.tensor_add(pte[:], Sap, Iap)
                nc.vector.tensor_scalar(Rap, pte[:], -1.0, 1.0, OP.mult, OP.add)

                nc.sync.dma_start(
                    yo_d[gbase : gbase + GR, :].rearrange(
                        "(s p t) c -> p s t c", s=GRP, p=128
                    ),
                    y_nat[:],
                )

    nc.compile()
    return nc


def _prep_consts(W1, b1, W2, b2):
    w1s = np.zeros((128, 128), np.float32)
    w2p = np.zeros((128, 32), np.float32)
    for q in range(4):
        w1s[32 * q : 32 * q + 8, 0:64] = W1
        w1s[32 * q + 8 : 32 * q + 16, 64:128] = W1
    w2p[0:64, 0] = W2[:, 0]
    w2p[64:128, 1] = W2[:, 0]
    b1p = np.concatenate([b1, b1]).reshape(128, 1).astype(np.float32)
    b2b = np.full((128, 1), b2[0], np.float32)
    ident = np.eye(128, dtype=np.float32)
    return w1s, w2p, b1p, b2b, ident


def run_sharded(y, x, W1, b1, W2, b2, trace=False, **spmd_kwargs):
    """Shard over 8 cores, run, gather. Returns (out, BassKernelResults)."""
    y = np.ascontiguousarray(np.asarray(y, np.float32))
    x = np.ascontiguousarray(np.asarray(x, np.float32))
    W1 = np.asarray(W1, np.float32)
    b1 = np.asarray(b1, np.float32)
    W2 = np.asarray(W2, np.float32)
    b2 = np.asarray(b2, np.float32)

    B = y.shape[0]
    RC = B // N_CORES
    key = RC
    if key not in _NC_CACHE:
        _NC_CACHE[key] = build_nc(RC)
    nc = _NC_CACHE[key]

    w1s, w2p, b1p, b2b, ident = _prep_consts(W1, b1, W2, b2)
    in_maps = []
    for c in range(N_CORES):
        in_maps.append(
            {
                "x": x[c * RC : (c + 1) * RC],
                "y": y[c * RC : (c + 1) * RC],
                "w1s": w1s,
                "w2p": w2p,
                "b1p": b1p,
                "b2b": b2b,
                "ident": ident,
            }
        )
    res = run_bass_kernel_spmd(
        nc, in_maps, core_ids=list(range(N_CORES)), trace=trace, **spmd_kwargs
    )
    out = np.concatenate([res.results[c]["yout"] for c in range(N_CORES)], axis=0)
    return out, res


def kernel(y, x, W1, b1, W2, b2):
    out, _ = run_sharded(y, x, W1, b1, W2, b2)
    return out


if __name__ == "__main__":
    rng = np.random.default_rng(0)
    B = N_CORES * 16384 * 8
    y0 = rng.random((B, 3), np.float32)
    y0 /= y0.sum(axis=1, keepdims=True)
    x = rng.random((B, 8), np.float32)
    W1 = (rng.standard_normal((8, 64)) * 0.3).astype(np.float32)
    b1 = np.zeros(64, np.float32)
    W2 = (rng.standard_normal((64, 1)) * 0.3).astype(np.float32)
    b2 = np.zeros(1, np.float32)
    out = kernel(y0, x, W1, b1, W2, b2)
    print(out[:4], out.shape)

